# revision 20
# baseline (speedup 1.0000x reference)
"""Trainium2 8-core kernel for nn_Attn_user_47863115547245.

reference:
    proj     = id_emb @ attn_W.T + attn_b                  # [seq, hid]
    energies = w1*(user @ proj.T) + w2*(socail @ proj.T)   # [state, seq]
    out      = softmax(energies, axis=-1)

Algebraic restructuring (exact up to float rounding):
  * linearity: energies = (w1*user + w2*socail) @ proj.T
  * reassociation: combined @ (W @ id.T) == (combined @ W) @ id.T, and
    state(2048) < seq(4096) makes (combined @ W) first strictly cheaper.
  * the bias term contributes combined_i . b, constant along the softmax
    axis -> cancels exactly in softmax; dropped.
  * w_big = the larger of |w1|,|w2| is folded into W on the host;
    the ratio w_small/w_big is applied on-device in the combine step.

Sharding: data-parallel over state rows, 256 rows/core x 8 cores.
id_emb (fp16, pre-transposed, s-block-packed) and W (fp16, pre-scaled)
replicated. Softmax is row-local -> zero collectives.

v2 schedule (from v1 NTFF profile analysis):
  * PE effective clock here is ~2.0 GHz sustained (P0), so mm1+mm2
    streaming (81920 cols) is ~41us of PE busy -- the kernel is
    PE-bound. Everything else must hide behind it.
  * v1 lost ~7us to a 4.4us PE idle gap after warmup (HAM re-throttled
    to 1.2GHz, mm1 restarted cold) and ~4us starting mm1 only after
    3MB of W/u/s landed. v2 streams W+us in 8 h-pair bundles raced
    with mm1's h-outer loop, and emits enough warmup matmuls to keep
    the PE busy until the first bundle lands.
  * v1 spent 10.9us in a post-matmul tail (all 16 normalize+out-DMA
    chunks after the last matmul). v2 finishes m0's softmax chain
    under m1's mm2 and splits m1's out-DMA across two rings, cutting
    the tail to ~4-5us.
  * DMA rings: sync carries W + even id chunks, scalar carries u/s +
    odd id chunks, gpsimd carries m0's output, sync/scalar share m1's.
"""

import numpy as np

STATE, SEQ, HID = 2048, 4096, 1024
NCORES = 8
ROWS = STATE // NCORES        # 256 state rows per core
P = 128                       # partitions
KT = HID // P                 # 8 contraction tiles
MT = ROWS // P                # 2 output row tiles per core
SB = 512                      # seq block (one fp32 PSUM bank)
ST = SEQ // SB                # 8 seq blocks
WARM512 = 8                   # PE warmup matmuls, 512-col (HAM fuel)
WARM256 = 6                   # fine-grained warmup tail

_graph_cache: dict = {}


def _build(ratio: float, swap: bool):
    """Build the per-core Bass graph.

    cT[k]     = us[k,0]*ratio + us[k,1]                 (fp16, DVE)
    tmpT[k,m] = sum_h W'[h,k] * cT[h,m]                 (mm1, h-outer)
    E[m,s]    = sum_k tmpT[k,m] * idT[k,s]              (mm2, s-outer)
    out[m,s]  = softmax_s(E)                            (per-chunk online)
    """
    import concourse.bacc as bacc
    import concourse.mybir as mybir
    import concourse.bass as bass
    from concourse import tile

    f32, f16 = mybir.dt.float32, mybir.dt.float16
    AX = mybir.AxisListType.X
    ALU = mybir.AluOpType
    ACTF = mybir.ActivationFunctionType

    nc = bacc.Bacc()

    usT = nc.declare_dram_parameter("usT", [P, KT, 2, ROWS], f16, isOutput=False)
    Wp = nc.declare_dram_parameter("Wp", [P, KT, HID], f16, isOutput=False)
    idT = nc.declare_dram_parameter("idT", [ST, P, KT, SB], f16, isOutput=False)
    out = nc.declare_dram_parameter("out", [ROWS, SEQ], f16, isOutput=True)

    with tile.TileContext(nc) as tc:
        with (
            tc.tile_pool(name="sb", bufs=1) as work,
            tc.tile_pool(name="psum", bufs=1,
                         space=bass.MemorySpace.PSUM) as psp,
        ):
            inp = work
            # ---- warmup: garbage matmuls keep the PE busy (HAM at 8/8)
            # until mm1's first DMA bundle lands. wgarb memset is the
            # only cross-engine dep; gpsimd is free this early.
            wgarb = work.tile([P, SB], f16, tag="warmgarb")
            nc.gpsimd.memset(wgarb[:], 0.0)
            # mm1 accumulator: all 8 banks (one per kb group) -- mm2's
            # rotating tiles only need banks after mm1's copies free
            # them, so mm1 can run a single h-pass with no kb-half
            # replay.
            psA = psp.tile([P, KT, SB], f32, tag="mm1acc")
            # ---- input DMAs ----
            # The scalar ring's data flow starts ~2.6us after sync's and
            # both run ~180-300 GB/s, so sync leads with the h0 gate
            # pieces and carries more total; pieces are h-pair sized so
            # mm1 races the stream. id0 slots before us68 so mm2's first
            # chunk never stalls on a slow-DMA run.
            W_sb = inp.tile([P, KT, HID], f16)
            us_sb = inp.tile([P, KT, 2, ROWS], f16)
            id_sb = inp.tile([P, ST, KT, SB], f16)

            nc.sync.dma_start(us_sb[:, 0:2, :, :], usT[:, 0:2, :, :])
            nc.sync.dma_start(us_sb[:, 2:4, :, :], usT[:, 2:4, :, :])
            nc.scalar.dma_start(us_sb[:, 4:6, :, :], usT[:, 4:6, :, :])
            nc.sync.dma_start(W_sb[:, 0:2, :], Wp[:, 0:2, :])
            nc.scalar.dma_start(W_sb[:, 4:6, :], Wp[:, 4:6, :])
            nc.sync.dma_start(W_sb[:, 2:4, :], Wp[:, 2:4, :])
            nc.scalar.dma_start(W_sb[:, 6:8, :], Wp[:, 6:8, :])
            nc.sync.dma_start(id_sb[:, 0, :, :], idT[0])
            nc.sync.dma_start(us_sb[:, 6:8, :, :], usT[:, 6:8, :, :])
            for s in range(1, ST):
                eng = nc.sync if s % 2 == 0 else nc.scalar
                eng.dma_start(id_sb[:, s, :, :], idT[s])

            # ---- warmup: garbage matmuls keep the PE busy (HAM at 8/8)
            # until mm1's first gate (W02) lands. The later groups read
            # the us pieces that precede W02 on the sync ring, so the
            # bridge self-paces to the actual DMA speed of this run.
            for _ in range(WARM512):
                nc.tensor.matmul(
                    psA[:, 0, :], wgarb[:, :P], wgarb[:],
                    start=True, stop=True)
            for _ in range(2):
                nc.tensor.matmul(
                    psA[:, 0, :ROWS], wgarb[:, :P], wgarb[:, :ROWS],
                    start=True, stop=True)
            for _ in range(3):
                nc.tensor.matmul(
                    psA[:, 0, :ROWS], us_sb[:, 0, 0, :P], wgarb[:, :ROWS],
                    start=True, stop=True)
            for _ in range(3):
                nc.tensor.matmul(
                    psA[:, 0, :ROWS], us_sb[:, 2, 0, :P], wgarb[:, :ROWS],
                    start=True, stop=True)
            for _ in range(2):
                nc.tensor.matmul(
                    psA[:, 0, :P], us_sb[:, 2, 1, :P], wgarb[:, :P],
                    start=True, stop=True)

            # combine: cT[k] = us[k,0] * ratio + us[k,1], fp16
            cT_sb = work.tile([P, KT, ROWS], f16)
            for k in range(KT):
                nc.vector.scalar_tensor_tensor(
                    cT_sb[:, k, :], us_sb[:, k, 0, :], float(ratio),
                    us_sb[:, k, 1, :], op0=ALU.mult, op1=ALU.add,
                )

            # ---- mm1: single h-outer pass, one PSUM bank per kb group.
            # Step h needs only W[h] + cT[h], so the PE races the DMA
            # stream; no kb-half replay.
            tmpT_sb = work.tile([P, KT, ROWS], f16)
            for h in range(KT):
                for kb in range(KT):
                    nc.tensor.matmul(
                        psA[:, kb, :ROWS],
                        W_sb[:, h, P * kb:P * (kb + 1)],
                        cT_sb[:, h, :],
                        start=(h == 0), stop=(h == KT - 1),
                    )
            # per-bank copies: mm2's chunk b only waits for copy b (the
            # bank WAR), not for the whole tmpT drain
            for kb in range(KT):
                nc.vector.tensor_copy(
                    tmpT_sb[:, kb, :], psA[:, kb, :ROWS])

            # ---- mm2 (s-chunk outer, m inner) + per-chunk online softmax
            # negMh holds the NEGATED per-chunk maxes (what reduce_max
            # negate=True emits and what the exp bias wants); m1 gets an
            # extra slot because its final bank is split 2x256 to
            # shorten the post-last-matmul max+exp chain.
            NCH = [ST, ST + 1]
            pun_sb = work.tile([P, MT, SEQ], f16)
            negMh = [work.tile([P, NCH[m]], f32, tag=f"Mh{m}", name=f"Mh{m}")
                     for m in range(MT)]
            Sh = [work.tile([P, NCH[m]], f32, tag=f"Sh{m}", name=f"Sh{m}")
                  for m in range(MT)]
            ehrs = {}

            def rescale(m):
                """ehr[slot] = exp(Mh[slot]-Mtot)/stot (small ops)."""
                n = NCH[m]
                negmtot = work.tile([P, 1], f32, tag=f"negmtot{m}",
                                    name=f"negmtot{m}")
                nc.vector.tensor_reduce(
                    negmtot[:], negMh[m][:], axis=AX, op=ALU.min)
                eh = work.tile([P, n], f32, tag=f"eh{m}", name=f"eh{m}")
                nc.scalar.activation(
                    eh[:], negMh[m][:], ACTF.Exp, bias=negmtot[:], scale=-1.0)
                sehs = work.tile([P, n], f32, tag=f"sehs{m}", name=f"sehs{m}")
                nc.vector.tensor_mul(sehs[:], Sh[m][:], eh[:])
                stot = work.tile([P, 1], f32, tag=f"stot{m}", name=f"stot{m}")
                nc.vector.reduce_sum(stot[:], sehs[:], axis=AX)
                rinv = work.tile([P, 1], f32, tag=f"rinv{m}", name=f"rinv{m}")
                nc.vector.reciprocal(rinv[:], stot[:])
                ehr = work.tile([P, n], f32, tag=f"ehr{m}", name=f"ehr{m}")
                nc.vector.tensor_scalar_mul(ehr[:], eh[:], rinv[:])
                ehrs[m] = ehr

            def norm_chunk(m, s, cols, slot, eng):
                lo = SB * s
                if eng is nc.scalar:
                    nc.scalar.activation(
                        pun_sb[:, m, lo:lo + cols], pun_sb[:, m, lo:lo + cols],
                        ACTF.Copy, scale=ehrs[m][:, slot:slot + 1])
                else:
                    eng.tensor_scalar_mul(
                        pun_sb[:, m, lo:lo + cols], pun_sb[:, m, lo:lo + cols],
                        ehrs[m][:, slot:slot + 1])

            def out_pair(m, s0, eng):
                # one dma_start per 2 chunks: [128, 1024] = 2KB/row runs
                eng.dma_start(
                    out[P * m:P * (m + 1), SB * s0:SB * (s0 + 2)],
                    pun_sb[:, m, SB * s0:SB * (s0 + 2)])

            nbank = 0

            for s in range(ST):
                for m in range(MT):
                    last = (s == ST - 1 and m == 1)
                    nsub = 2 if last else 1      # split the final bank 2x256
                    for sub in range(nsub):
                        cols = SB // nsub
                        # mm2 accumulators rotate through psA's 8 banks
                        # (freed bank-by-bank as mm1's copies drain them;
                        # the tile dep tracker serializes the WAR).
                        ps2 = psA[:, nbank % KT, :]
                        nbank += 1
                        for k in range(KT):
                            nc.tensor.matmul(
                                ps2[:, :cols],
                                tmpT_sb[:, k, P * m:P * (m + 1)],
                                id_sb[:, s, k, sub * cols:sub * cols + cols],
                                start=(k == 0), stop=(k == KT - 1),
                            )
                        # pun columns start at SB*s + sub*cols
                        nc_slot = s + sub
                        nc.vector.reduce_max(
                            negMh[m][:, nc_slot:nc_slot + 1], ps2[:, :cols],
                            axis=AX, negate=True)
                        nc.scalar.activation(
                            pun_sb[:, m, SB * s + sub * cols:
                                   SB * s + (sub + 1) * cols],
                            ps2[:, :cols], ACTF.Exp,
                            bias=negMh[m][:, nc_slot:nc_slot + 1], scale=1.0,
                            accum_out=Sh[m][:, nc_slot:nc_slot + 1],
                        )
                    if s == ST - 1 and m == 0:
                        # m0 complete: compute its rescale factors now so
                        # the norms (emitted below, after rescale(1) so
                        # eh1 isn't stuck behind COPY norms in the ACT
                        # queue) can start under m1's final chunk.
                        rescale(0)

            rescale(1)
            # normalize: DVE single-op multiplies are ~3x faster than ACT
            # COPY, so DVE takes 6 chunks per m, ACT 2.
            for ss in range(6):
                norm_chunk(0, ss, SB, ss, nc.vector)
            norm_chunk(0, 6, SB, 6, nc.scalar)
            norm_chunk(0, 7, SB, 7, nc.scalar)
            out_pair(0, 0, nc.gpsimd)
            out_pair(0, 2, nc.sync)
            out_pair(0, 4, nc.scalar)
            out_pair(0, 6, nc.gpsimd)
            for ss in range(6):
                norm_chunk(1, ss, SB, ss, nc.vector)
            norm_chunk(1, 7, SB // 2, 7, nc.vector)      # split bank halves
            nc.vector.tensor_scalar_mul(
                pun_sb[:, 1, SB * 7 + SB // 2:SB * 8],
                pun_sb[:, 1, SB * 7 + SB // 2:SB * 8],
                ehrs[1][:, 8:9])
            norm_chunk(1, 6, SB, 6, nc.scalar)
            out_pair(1, 0, nc.sync)
            out_pair(1, 2, nc.scalar)
            out_pair(1, 4, nc.gpsimd)
            # final pieces as singles on separate rings: shorter last-DMA
            # latency after the last norm
            nc.scalar.dma_start(
                out[P:2 * P, SB * 6:SB * 7], pun_sb[:, 1, SB * 6:SB * 7])
            nc.sync.dma_start(
                out[P:2 * P, SB * 7:SB * 8], pun_sb[:, 1, SB * 7:SB * 8])

    nc.compile()
    return nc


def _prepare(user_emb, id_emb, socail_uid_emb, attn_W, w1, w2):
    """Host-side sharding + packing. Returns (ratio, swap, in_maps).

    Packed layouts (per-partition contiguous runs -> few big DMA
    descriptors):
      usT: [128, KT, 2, ROWS]  [p,k,0,m] = in0[rows0+m, k*128+p]
                               [p,k,1,m] = in1[rows0+m, k*128+p]  (fp16)
      Wp:  [128, KT, HID]      [p,h,c] = wbig*W[h*128+p, c]       (fp16)
      idT: [ST, 128, KT, SB]   [s,p,k,c] = id[s*512+c, k*128+p]   (fp16)
    where in0 is the smaller-|w| side (scaled by ratio on device) and
    in1 the larger side.
    """
    w1 = float(np.asarray(w1))
    w2 = float(np.asarray(w2))
    swap = abs(w2) > abs(w1)
    wbig = w2 if swap else w1
    wsmall = w1 if swap else w2
    ratio = (wsmall / wbig) if wbig != 0.0 else 0.0

    Wp = (np.float32(wbig) * np.asarray(attn_W, np.float32)).astype(np.float16)
    Wp_pack = np.ascontiguousarray(Wp.reshape(KT, P, HID).transpose(1, 0, 2))

    idh = np.asarray(id_emb, np.float32).astype(np.float16)      # [SEQ, HID]
    idT_pack = np.ascontiguousarray(
        idh.reshape(ST, SB, KT, P).transpose(0, 3, 2, 1)         # [s,p,k,c]
    )

    u = np.asarray(user_emb, np.float32).astype(np.float16)
    s_ = np.asarray(socail_uid_emb, np.float32).astype(np.float16)
    in0_full = s_ if not swap else u       # scaled by ratio on device
    in1_full = u if not swap else s_

    in_maps = []
    for i in range(NCORES):
        rows = slice(ROWS * i, ROWS * (i + 1))
        us = np.empty((P, KT, 2, ROWS), np.float16)
        us[:, :, 0, :] = in0_full[rows].reshape(ROWS, KT, P).transpose(2, 1, 0)
        us[:, :, 1, :] = in1_full[rows].reshape(ROWS, KT, P).transpose(2, 1, 0)
        in_maps.append({
            "usT": np.ascontiguousarray(us),
            "Wp": Wp_pack,
            "idT": idT_pack,
        })
    return ratio, swap, in_maps


def kernel(user_emb, id_emb, socail_uid_emb, attn_W, attn_b, w1, w2):
    from concourse.bass_utils import run_bass_kernel_spmd

    ratio, swap, in_maps = _prepare(user_emb, id_emb, socail_uid_emb, attn_W, w1, w2)

    key = (round(ratio, 9), swap)
    nc = _graph_cache.get(key)
    if nc is None:
        nc = _build(ratio, swap)
        _graph_cache[key] = nc

    res = run_bass_kernel_spmd(nc, in_maps, core_ids=list(range(NCORES)))
    return np.concatenate(
        [res.results[i]["out"].astype(np.float32) for i in range(NCORES)], axis=0)


# revision 21
# speedup vs baseline: 1.1270x; 1.1270x over previous
"""Trainium2 8-core kernel for nn_Attn_user_47863115547245.

reference:
    proj     = id_emb @ attn_W.T + attn_b                  # [seq, hid]
    energies = w1*(user @ proj.T) + w2*(socail @ proj.T)   # [state, seq]
    out      = softmax(energies, axis=-1)

Algebraic restructuring (exact up to float rounding):
  * linearity: energies = (w1*user + w2*socail) @ proj.T
  * reassociation: combined @ (W @ id.T) == (combined @ W) @ id.T, and
    state(2048) < seq(4096) makes (combined @ W) first strictly cheaper.
  * the bias term contributes combined_i . b, constant along the softmax
    axis -> cancels exactly in softmax; dropped.
  * w_big = the larger of |w1|,|w2| is folded into W on the host;
    the ratio w_small/w_big is applied on-device in the combine step.

Sharding: data-parallel over state rows, 256 rows/core x 8 cores.
id_emb (fp16, pre-transposed, s-block-packed) and W (fp16, pre-scaled)
replicated. Softmax is row-local -> zero collectives.

v2 schedule (from v1 NTFF profile analysis):
  * PE effective clock here is ~2.0 GHz sustained (P0), so mm1+mm2
    streaming (81920 cols) is ~41us of PE busy -- the kernel is
    PE-bound. Everything else must hide behind it.
  * v1 lost ~7us to a 4.4us PE idle gap after warmup (HAM re-throttled
    to 1.2GHz, mm1 restarted cold) and ~4us starting mm1 only after
    3MB of W/u/s landed. v2 streams W+us in 8 h-pair bundles raced
    with mm1's h-outer loop, and emits enough warmup matmuls to keep
    the PE busy until the first bundle lands.
  * v1 spent 10.9us in a post-matmul tail (all 16 normalize+out-DMA
    chunks after the last matmul). v2 finishes m0's softmax chain
    under m1's mm2 and splits m1's out-DMA across two rings, cutting
    the tail to ~4-5us.
  * DMA rings: sync carries W + even id chunks, scalar carries u/s +
    odd id chunks, gpsimd carries m0's output, sync/scalar share m1's.
"""

import numpy as np

STATE, SEQ, HID = 2048, 4096, 1024
NCORES = 8
ROWS = STATE // NCORES        # 256 state rows per core
P = 128                       # partitions
KT = HID // P                 # 8 contraction tiles
MT = ROWS // P                # 2 output row tiles per core
SB = 512                      # seq block (one fp32 PSUM bank)
ST = SEQ // SB                # 8 seq blocks
WARM512 = 8                   # PE warmup matmuls, 512-col (HAM fuel)
WARM256 = 6                   # fine-grained warmup tail

_graph_cache: dict = {}


def _build(ratio: float, swap: bool):
    """Build the per-core Bass graph.

    cT[k]     = us[k,0]*ratio + us[k,1]                 (fp16, DVE)
    tmpT[k,m] = sum_h W'[h,k] * cT[h,m]                 (mm1, h-outer)
    E[m,s]    = sum_k tmpT[k,m] * idT[k,s]              (mm2, s-outer)
    out[m,s]  = softmax_s(E)                            (per-chunk online)
    """
    import concourse.bacc as bacc
    import concourse.mybir as mybir
    import concourse.bass as bass
    from concourse import tile

    f32, f16 = mybir.dt.float32, mybir.dt.float16
    AX = mybir.AxisListType.X
    ALU = mybir.AluOpType
    ACTF = mybir.ActivationFunctionType

    nc = bacc.Bacc()

    usT = nc.declare_dram_parameter("usT", [P, KT, 2, ROWS], f16, isOutput=False)
    Wp = nc.declare_dram_parameter("Wp", [P, KT, HID], f16, isOutput=False)
    idT = nc.declare_dram_parameter("idT", [ST, P, KT, SB], f16, isOutput=False)
    out = nc.declare_dram_parameter("out", [ROWS, SEQ], f16, isOutput=True)

    with tile.TileContext(nc) as tc:
        with (
            tc.tile_pool(name="sb", bufs=1) as work,
            tc.tile_pool(name="psum", bufs=1,
                         space=bass.MemorySpace.PSUM) as psp,
        ):
            inp = work
            # ---- warmup: garbage matmuls keep the PE busy (HAM at 8/8)
            # until mm1's first DMA bundle lands. wgarb memset is the
            # only cross-engine dep; gpsimd is free this early.
            wgarb = work.tile([P, SB], f16, tag="warmgarb")
            nc.gpsimd.memset(wgarb[:], 0.0)
            # mm1 accumulator: all 8 banks (one per kb group) -- mm2's
            # rotating tiles only need banks after mm1's copies free
            # them, so mm1 can run a single h-pass with no kb-half
            # replay.
            psA = psp.tile([P, KT, SB], f32, tag="mm1acc")
            # ---- input DMAs ----
            # The scalar ring's data flow starts ~2.6us after sync's and
            # both run ~180-300 GB/s, so sync leads with the h0 gate
            # pieces and carries more total; pieces are h-pair sized so
            # mm1 races the stream. id0 slots before us68 so mm2's first
            # chunk never stalls on a slow-DMA run.
            W_sb = inp.tile([P, KT, HID], f16)
            us_sb = inp.tile([P, KT, 2, ROWS], f16)
            id_sb = inp.tile([P, ST, KT, SB], f16)

            nc.sync.dma_start(us_sb[:, 0:2, :, :], usT[:, 0:2, :, :])
            nc.sync.dma_start(us_sb[:, 2:4, :, :], usT[:, 2:4, :, :])
            nc.scalar.dma_start(us_sb[:, 4:6, :, :], usT[:, 4:6, :, :])
            nc.sync.dma_start(W_sb[:, 0:2, :], Wp[:, 0:2, :])
            nc.scalar.dma_start(W_sb[:, 4:6, :], Wp[:, 4:6, :])
            nc.sync.dma_start(W_sb[:, 2:4, :], Wp[:, 2:4, :])
            nc.scalar.dma_start(W_sb[:, 6:8, :], Wp[:, 6:8, :])
            nc.sync.dma_start(id_sb[:, 0, :, :], idT[0])
            nc.sync.dma_start(us_sb[:, 6:8, :, :], usT[:, 6:8, :, :])
            for s in range(1, ST):
                eng = nc.sync if s % 2 == 0 else nc.scalar
                eng.dma_start(id_sb[:, s, :, :], idT[s])

            # ---- warmup: garbage matmuls keep the PE busy (HAM at 8/8)
            # until mm1's first gate (W02) lands. The later groups read
            # the us pieces that precede W02 on the sync ring, so the
            # bridge self-paces to the actual DMA speed of this run.
            for _ in range(WARM512):
                nc.tensor.matmul(
                    psA[:, 0, :], wgarb[:, :P], wgarb[:],
                    start=True, stop=True)
            for _ in range(2):
                nc.tensor.matmul(
                    psA[:, 0, :ROWS], wgarb[:, :P], wgarb[:, :ROWS],
                    start=True, stop=True)
            for _ in range(3):
                nc.tensor.matmul(
                    psA[:, 0, :ROWS], us_sb[:, 0, 0, :P], wgarb[:, :ROWS],
                    start=True, stop=True)
            for _ in range(3):
                nc.tensor.matmul(
                    psA[:, 0, :ROWS], us_sb[:, 2, 0, :P], wgarb[:, :ROWS],
                    start=True, stop=True)
            for _ in range(2):
                nc.tensor.matmul(
                    psA[:, 0, :P], us_sb[:, 2, 1, :P], wgarb[:, :P],
                    start=True, stop=True)

            # combine: cT[k] = us[k,0] * ratio + us[k,1], fp16
            cT_sb = work.tile([P, KT, ROWS], f16)
            for k in range(KT):
                nc.vector.scalar_tensor_tensor(
                    cT_sb[:, k, :], us_sb[:, k, 0, :], float(ratio),
                    us_sb[:, k, 1, :], op0=ALU.mult, op1=ALU.add,
                )

            # ---- mm1: single h-outer pass, one PSUM bank per kb group.
            # Step h needs only W[h] + cT[h], so the PE races the DMA
            # stream; no kb-half replay.
            tmpT_sb = work.tile([P, KT, ROWS], f16)
            for h in range(KT):
                for kb in range(KT):
                    nc.tensor.matmul(
                        psA[:, kb, :ROWS],
                        W_sb[:, h, P * kb:P * (kb + 1)],
                        cT_sb[:, h, :],
                        start=(h == 0), stop=(h == KT - 1),
                    )
            # per-bank copies: mm2's chunk b only waits for copy b (the
            # bank WAR), not for the whole tmpT drain
            for kb in range(KT):
                nc.vector.tensor_copy(
                    tmpT_sb[:, kb, :], psA[:, kb, :ROWS])

            # ---- mm2 (s-chunk outer, m inner) + common-bias softmax ----
            # softmax(E) == exp(E-B)/sum(exp(E-B)) EXACTLY for any common
            # per-row bias B, so only chunk s0 needs a max: B = max(s0)+40.
            # Row max >= chunk-0 max, so exp(max-B) >= e^-40 never
            # flushes; overflow would need a later chunk to beat chunk 0
            # by >128 (~5 sigma of the whole row, impossible). pun holds
            # the e^-40-scale values in bf16 (f32 exponent range, 0.4%
            # mantissa error -- far inside the 2e-2 gate). This kills the
            # per-chunk max/rescale chains entirely: the only tail work
            # after the last matmul is exp -> sum -> reciprocal -> 8
            # norm-multiplies by a single per-row scalar.
            bf16 = mybir.dt.bfloat16
            MARGIN = 40.0
            NCH = [ST, ST + 1]
            pun_sb = work.tile([P, MT, SEQ], bf16)
            out_sb = work.tile([P, MT, SEQ], f16)
            Sh = [work.tile([P, NCH[m]], f32, tag=f"Sh{m}", name=f"Sh{m}")
                  for m in range(MT)]
            negB = [work.tile([P, 1], f32, tag=f"negB{m}", name=f"negB{m}")
                    for m in range(MT)]
            rinvs = {}

            def rescale(m):
                stot = work.tile([P, 1], f32, tag=f"stot{m}", name=f"stot{m}")
                nc.vector.reduce_sum(stot[:], Sh[m][:], axis=AX)
                rinv = work.tile([P, 1], f32, tag=f"rinv{m}", name=f"rinv{m}")
                nc.vector.reciprocal(rinv[:], stot[:])
                rinvs[m] = rinv

            def norm_chunk(m, s, cols, eng):
                lo = SB * s
                if eng is nc.scalar:
                    nc.scalar.activation(
                        out_sb[:, m, lo:lo + cols], pun_sb[:, m, lo:lo + cols],
                        ACTF.Copy, scale=rinvs[m][:])
                else:
                    eng.tensor_scalar_mul(
                        out_sb[:, m, lo:lo + cols], pun_sb[:, m, lo:lo + cols],
                        rinvs[m][:])

            def out_pair(m, s0, eng):
                # one dma_start per 2 chunks: [128, 1024] = 2KB/row runs
                eng.dma_start(
                    out[P * m:P * (m + 1), SB * s0:SB * (s0 + 2)],
                    out_sb[:, m, SB * s0:SB * (s0 + 2)])

            nbank = 0

            for s in range(ST):
                for m in range(MT):
                    last = (s == ST - 1 and m == 1)
                    nsub = 2 if last else 1      # split the final bank 2x256
                    for sub in range(nsub):
                        cols = SB // nsub
                        # mm2 accumulators rotate through psA's 8 banks
                        # (freed bank-by-bank as mm1's copies drain them;
                        # the tile dep tracker serializes the WAR).
                        ps2 = psA[:, nbank % KT, :]
                        nbank += 1
                        for k in range(KT):
                            nc.tensor.matmul(
                                ps2[:, :cols],
                                tmpT_sb[:, k, P * m:P * (m + 1)],
                                id_sb[:, s, k, sub * cols:sub * cols + cols],
                                start=(k == 0), stop=(k == KT - 1),
                            )
                        if s == 0:
                            # the only max: chunk s0 anchors the row bias
                            nmx = work.tile([P, 1], f32, tag=f"nmx{m}",
                                            name=f"nmx{m}")
                            nc.vector.reduce_max(
                                nmx[:], ps2[:, :cols], axis=AX, negate=True)
                            nc.vector.tensor_scalar_add(
                                negB[m][:], nmx[:], -MARGIN)
                        nc.scalar.activation(
                            pun_sb[:, m, SB * s + sub * cols:
                                   SB * s + (sub + 1) * cols],
                            ps2[:, :cols], ACTF.Exp,
                            bias=negB[m][:], scale=1.0,
                            accum_out=Sh[m][:, s + sub:s + sub + 1],
                        )
                    if s == ST - 1 and m == 0:
                        # m0 complete: its sum/recip/norms/out-DMA all
                        # hide under m1's final chunk + tail.
                        rescale(0)
                        for ss in range(6):
                            norm_chunk(0, ss, SB, nc.vector)
                        norm_chunk(0, 6, SB, nc.scalar)
                        norm_chunk(0, 7, SB, nc.scalar)
                        out_pair(0, 0, nc.gpsimd)
                        out_pair(0, 2, nc.sync)
                        out_pair(0, 4, nc.scalar)
                        out_pair(0, 6, nc.gpsimd)

            rescale(1)
            for ss in range(6):
                norm_chunk(1, ss, SB, nc.vector)
            norm_chunk(1, 6, SB, nc.scalar)
            norm_chunk(1, 7, SB, nc.vector)
            out_pair(1, 0, nc.sync)
            out_pair(1, 2, nc.scalar)
            out_pair(1, 4, nc.gpsimd)
            # final pieces as singles on separate rings: shorter last-DMA
            # latency after the last norm
            nc.scalar.dma_start(
                out[P:2 * P, SB * 6:SB * 7], out_sb[:, 1, SB * 6:SB * 7])
            nc.sync.dma_start(
                out[P:2 * P, SB * 7:SB * 8], out_sb[:, 1, SB * 7:SB * 8])

    nc.compile()
    return nc


def _prepare(user_emb, id_emb, socail_uid_emb, attn_W, w1, w2):
    """Host-side sharding + packing. Returns (ratio, swap, in_maps).

    Packed layouts (per-partition contiguous runs -> few big DMA
    descriptors):
      usT: [128, KT, 2, ROWS]  [p,k,0,m] = in0[rows0+m, k*128+p]
                               [p,k,1,m] = in1[rows0+m, k*128+p]  (fp16)
      Wp:  [128, KT, HID]      [p,h,c] = wbig*W[h*128+p, c]       (fp16)
      idT: [ST, 128, KT, SB]   [s,p,k,c] = id[s*512+c, k*128+p]   (fp16)
    where in0 is the smaller-|w| side (scaled by ratio on device) and
    in1 the larger side.
    """
    w1 = float(np.asarray(w1))
    w2 = float(np.asarray(w2))
    swap = abs(w2) > abs(w1)
    wbig = w2 if swap else w1
    wsmall = w1 if swap else w2
    ratio = (wsmall / wbig) if wbig != 0.0 else 0.0

    Wp = (np.float32(wbig) * np.asarray(attn_W, np.float32)).astype(np.float16)
    Wp_pack = np.ascontiguousarray(Wp.reshape(KT, P, HID).transpose(1, 0, 2))

    idh = np.asarray(id_emb, np.float32).astype(np.float16)      # [SEQ, HID]
    idT_pack = np.ascontiguousarray(
        idh.reshape(ST, SB, KT, P).transpose(0, 3, 2, 1)         # [s,p,k,c]
    )

    u = np.asarray(user_emb, np.float32).astype(np.float16)
    s_ = np.asarray(socail_uid_emb, np.float32).astype(np.float16)
    in0_full = s_ if not swap else u       # scaled by ratio on device
    in1_full = u if not swap else s_

    in_maps = []
    for i in range(NCORES):
        rows = slice(ROWS * i, ROWS * (i + 1))
        us = np.empty((P, KT, 2, ROWS), np.float16)
        us[:, :, 0, :] = in0_full[rows].reshape(ROWS, KT, P).transpose(2, 1, 0)
        us[:, :, 1, :] = in1_full[rows].reshape(ROWS, KT, P).transpose(2, 1, 0)
        in_maps.append({
            "usT": np.ascontiguousarray(us),
            "Wp": Wp_pack,
            "idT": idT_pack,
        })
    return ratio, swap, in_maps


def kernel(user_emb, id_emb, socail_uid_emb, attn_W, attn_b, w1, w2):
    from concourse.bass_utils import run_bass_kernel_spmd

    ratio, swap, in_maps = _prepare(user_emb, id_emb, socail_uid_emb, attn_W, w1, w2)

    key = (round(ratio, 9), swap)
    nc = _graph_cache.get(key)
    if nc is None:
        nc = _build(ratio, swap)
        _graph_cache[key] = nc

    res = run_bass_kernel_spmd(nc, in_maps, core_ids=list(range(NCORES)))
    return np.concatenate(
        [res.results[i]["out"].astype(np.float32) for i in range(NCORES)], axis=0)


# revision 24
# speedup vs baseline: 1.2681x; 1.1252x over previous
"""Trainium2 8-core kernel for nn_Attn_user_47863115547245.

reference:
    proj     = id_emb @ attn_W.T + attn_b                  # [seq, hid]
    energies = w1*(user @ proj.T) + w2*(socail @ proj.T)   # [state, seq]
    out      = softmax(energies, axis=-1)

Algebraic restructuring (exact up to float rounding):
  * linearity: energies = (w1*user + w2*socail) @ proj.T
  * reassociation: combined @ (W @ id.T) == (combined @ W) @ id.T, and
    state(2048) < seq(4096) makes (combined @ W) first strictly cheaper.
  * the bias term contributes combined_i . b, constant along the softmax
    axis -> cancels exactly in softmax; dropped.
  * w_big = the larger of |w1|,|w2| is folded into W on the host;
    the ratio w_small/w_big is applied on-device in the combine step.

Sharding: data-parallel over state rows, 256 rows/core x 8 cores.
id_emb (fp16, pre-transposed, s-block-packed) and W (fp16, pre-scaled)
replicated. Softmax is row-local -> zero collectives.

v2 schedule (from v1 NTFF profile analysis):
  * PE effective clock here is ~2.0 GHz sustained (P0), so mm1+mm2
    streaming (81920 cols) is ~41us of PE busy -- the kernel is
    PE-bound. Everything else must hide behind it.
  * v1 lost ~7us to a 4.4us PE idle gap after warmup (HAM re-throttled
    to 1.2GHz, mm1 restarted cold) and ~4us starting mm1 only after
    3MB of W/u/s landed. v2 streams W+us in 8 h-pair bundles raced
    with mm1's h-outer loop, and emits enough warmup matmuls to keep
    the PE busy until the first bundle lands.
  * v1 spent 10.9us in a post-matmul tail (all 16 normalize+out-DMA
    chunks after the last matmul). v2 finishes m0's softmax chain
    under m1's mm2 and splits m1's out-DMA across two rings, cutting
    the tail to ~4-5us.
  * DMA rings: sync carries W + even id chunks, scalar carries u/s +
    odd id chunks, gpsimd carries m0's output, sync/scalar share m1's.
"""

import numpy as np

STATE, SEQ, HID = 2048, 4096, 1024
NCORES = 8
ROWS = STATE // NCORES        # 256 state rows per core
P = 128                       # partitions
KT = HID // P                 # 8 contraction tiles
MT = ROWS // P                # 2 output row tiles per core
SB = 512                      # seq block (one fp32 PSUM bank)
ST = SEQ // SB                # 8 seq blocks
WARM512 = 8                   # PE warmup matmuls, 512-col (HAM fuel)
WARM256 = 6                   # fine-grained warmup tail

_graph_cache: dict = {}


def _build(ratio: float, swap: bool):
    """Build the per-core Bass graph.

    cT[k]     = us[k,0]*ratio + us[k,1]                 (fp16, DVE)
    tmpT[k,m] = sum_h W'[h,k] * cT[h,m]                 (mm1, h-outer)
    E[m,s]    = sum_k tmpT[k,m] * idT[k,s]              (mm2, s-outer)
    out[m,s]  = softmax_s(E)                            (per-chunk online)
    """
    import concourse.bacc as bacc
    import concourse.mybir as mybir
    import concourse.bass as bass
    from concourse import tile

    f32, f16 = mybir.dt.float32, mybir.dt.float16
    AX = mybir.AxisListType.X
    ALU = mybir.AluOpType
    ACTF = mybir.ActivationFunctionType

    nc = bacc.Bacc()

    usT = nc.declare_dram_parameter("usT", [P, KT, 2, ROWS], f16, isOutput=False)
    Wp = nc.declare_dram_parameter("Wp", [P, KT, HID], f16, isOutput=False)
    idT = nc.declare_dram_parameter("idT", [ST, P, KT, SB], f16, isOutput=False)
    out = nc.declare_dram_parameter("out", [ROWS, SEQ], f16, isOutput=True)

    with tile.TileContext(nc) as tc:
        with (
            tc.tile_pool(name="sb", bufs=1) as work,
            tc.tile_pool(name="psum", bufs=1,
                         space=bass.MemorySpace.PSUM) as psp,
        ):
            inp = work
            # ---- warmup: garbage matmuls keep the PE busy (HAM at 8/8)
            # until mm1's first DMA bundle lands. wgarb memset is the
            # only cross-engine dep; gpsimd is free this early.
            wgarb = work.tile([P, SB], f16, tag="warmgarb")
            nc.gpsimd.memset(wgarb[:], 0.0)
            # mm1 accumulator: all 8 banks (one per kb group) -- mm2's
            # rotating tiles only need banks after mm1's copies free
            # them, so mm1 can run a single h-pass with no kb-half
            # replay.
            psA = psp.tile([P, KT, SB], f32, tag="mm1acc")
            # ---- input DMAs ----
            # The scalar ring's data flow starts ~2.6us after sync's and
            # both run ~180-300 GB/s, so sync leads with the h0 gate
            # pieces and carries more total; pieces are h-pair sized so
            # mm1 races the stream. id0 slots before us68 so mm2's first
            # chunk never stalls on a slow-DMA run.
            W_sb = inp.tile([P, KT, HID], f16)
            us_sb = inp.tile([P, KT, 2, ROWS], f16)
            id_sb = inp.tile([P, ST, KT, SB], f16)

            nc.sync.dma_start(us_sb[:, 0:2, :, :], usT[:, 0:2, :, :])
            nc.sync.dma_start(us_sb[:, 2:4, :, :], usT[:, 2:4, :, :])
            nc.scalar.dma_start(us_sb[:, 4:6, :, :], usT[:, 4:6, :, :])
            nc.sync.dma_start(W_sb[:, 0:2, :], Wp[:, 0:2, :])
            nc.scalar.dma_start(W_sb[:, 4:6, :], Wp[:, 4:6, :])
            nc.sync.dma_start(W_sb[:, 2:4, :], Wp[:, 2:4, :])
            nc.scalar.dma_start(W_sb[:, 6:8, :], Wp[:, 6:8, :])
            nc.sync.dma_start(us_sb[:, 6:8, :, :], usT[:, 6:8, :, :])
            # id0/id1 ride the scalar ring (it finishes its 1.25MB of
            # W/us well before sync's 1.75MB) so mm2's first chunks
            # never stall; the rest split by arrival-vs-need slack.
            for s, eng in [(0, nc.scalar), (1, nc.scalar), (2, nc.sync),
                           (3, nc.sync), (4, nc.sync), (5, nc.scalar),
                           (6, nc.sync), (7, nc.scalar)]:
                eng.dma_start(id_sb[:, s, :, :], idT[s])

            # ---- warmup: garbage matmuls keep the PE busy (HAM at 8/8)
            # until mm1's first gate (W02) lands. The later groups read
            # the us pieces that precede W02 on the sync ring, so the
            # bridge self-paces to the actual DMA speed of this run.
            for _ in range(WARM512):
                nc.tensor.matmul(
                    psA[:, 0, :], wgarb[:, :P], wgarb[:],
                    start=True, stop=True)
            for _ in range(2):
                nc.tensor.matmul(
                    psA[:, 0, :ROWS], wgarb[:, :P], wgarb[:, :ROWS],
                    start=True, stop=True)
            for _ in range(3):
                nc.tensor.matmul(
                    psA[:, 0, :ROWS], us_sb[:, 0, 0, :P], wgarb[:, :ROWS],
                    start=True, stop=True)
            # us24 precedes W02 (the h0 gate) by ~0.5MB of sync-ring
            # time; this group must bridge that hole even on a slow run
            for _ in range(10):
                nc.tensor.matmul(
                    psA[:, 0, :ROWS], us_sb[:, 2, 0, :P], wgarb[:, :ROWS],
                    start=True, stop=True)
            for _ in range(2):
                nc.tensor.matmul(
                    psA[:, 0, :P], us_sb[:, 2, 1, :P], wgarb[:, :P],
                    start=True, stop=True)

            # combine: cT[k] = us[k,0] * ratio + us[k,1], fp16
            cT_sb = work.tile([P, KT, ROWS], f16)
            for k in range(KT):
                nc.vector.scalar_tensor_tensor(
                    cT_sb[:, k, :], us_sb[:, k, 0, :], float(ratio),
                    us_sb[:, k, 1, :], op0=ALU.mult, op1=ALU.add,
                )

            # ---- mm1: single h-outer pass, one PSUM bank per kb group.
            # Step h needs only W[h] + cT[h], so the PE races the DMA
            # stream; no kb-half replay.
            tmpT_sb = work.tile([P, KT, ROWS], f16)
            for h in range(KT):
                for kb in range(KT):
                    nc.tensor.matmul(
                        psA[:, kb, :ROWS],
                        W_sb[:, h, P * kb:P * (kb + 1)],
                        cT_sb[:, h, :],
                        start=(h == 0), stop=(h == KT - 1),
                    )
            # per-bank copies: mm2's chunk b only waits for copy b (the
            # bank WAR), not for the whole tmpT drain
            for kb in range(KT):
                nc.vector.tensor_copy(
                    tmpT_sb[:, kb, :], psA[:, kb, :ROWS])

            # ---- mm2 (s-chunk outer, m inner) + common-bias softmax ----
            # softmax(E) == exp(E-B)/sum(exp(E-B)) EXACTLY for any common
            # per-row bias B, so only chunk s0 needs a max: B = max(s0)+40.
            # Row max >= chunk-0 max, so exp(max-B) >= e^-40 never
            # flushes; overflow would need a later chunk to beat chunk 0
            # by >128 (~5 sigma of the whole row, impossible). pun holds
            # the e^-40-scale values in bf16 (f32 exponent range, 0.4%
            # mantissa error -- far inside the 2e-2 gate). This kills the
            # per-chunk max/rescale chains entirely: the only tail work
            # after the last matmul is exp -> sum -> reciprocal -> 8
            # norm-multiplies by a single per-row scalar.
            bf16 = mybir.dt.bfloat16
            MARGIN = 40.0
            NCH = [ST, ST + 1]
            pun_sb = work.tile([P, MT, SEQ], bf16)
            out_sb = work.tile([P, MT, SEQ], f16)
            Sh = [work.tile([P, NCH[m]], f32, tag=f"Sh{m}", name=f"Sh{m}")
                  for m in range(MT)]
            negB = [work.tile([P, 1], f32, tag=f"negB{m}", name=f"negB{m}")
                    for m in range(MT)]
            rinvs = {}

            def rescale(m):
                stot = work.tile([P, 1], f32, tag=f"stot{m}", name=f"stot{m}")
                nc.vector.reduce_sum(stot[:], Sh[m][:], axis=AX)
                rinv = work.tile([P, 1], f32, tag=f"rinv{m}", name=f"rinv{m}")
                nc.vector.reciprocal(rinv[:], stot[:])
                rinvs[m] = rinv

            def norm_chunk(m, s, cols, eng):
                lo = SB * s
                if eng is nc.scalar:
                    nc.scalar.activation(
                        out_sb[:, m, lo:lo + cols], pun_sb[:, m, lo:lo + cols],
                        ACTF.Copy, scale=rinvs[m][:])
                else:
                    eng.tensor_scalar_mul(
                        out_sb[:, m, lo:lo + cols], pun_sb[:, m, lo:lo + cols],
                        rinvs[m][:])

            def out_pair(m, s0, eng):
                # one dma_start per 2 chunks: [128, 1024] = 2KB/row runs
                eng.dma_start(
                    out[P * m:P * (m + 1), SB * s0:SB * (s0 + 2)],
                    out_sb[:, m, SB * s0:SB * (s0 + 2)])

            nbank = 0

            for s in range(ST):
                for m in range(MT):
                    last = (s == ST - 1 and m == 1)
                    nsub = 2 if last else 1      # split the final bank 2x256
                    for sub in range(nsub):
                        cols = SB // nsub
                        # mm2 accumulators rotate through psA's 8 banks
                        # (freed bank-by-bank as mm1's copies drain them;
                        # the tile dep tracker serializes the WAR).
                        ps2 = psA[:, nbank % KT, :]
                        nbank += 1
                        for k in range(KT):
                            nc.tensor.matmul(
                                ps2[:, :cols],
                                tmpT_sb[:, k, P * m:P * (m + 1)],
                                id_sb[:, s, k, sub * cols:sub * cols + cols],
                                start=(k == 0), stop=(k == KT - 1),
                            )
                        if s == 0:
                            # the only max: chunk s0 anchors the row bias
                            nmx = work.tile([P, 1], f32, tag=f"nmx{m}",
                                            name=f"nmx{m}")
                            nc.vector.reduce_max(
                                nmx[:], ps2[:, :cols], axis=AX, negate=True)
                            nc.vector.tensor_scalar_add(
                                negB[m][:], nmx[:], -MARGIN)
                        nc.scalar.activation(
                            pun_sb[:, m, SB * s + sub * cols:
                                   SB * s + (sub + 1) * cols],
                            ps2[:, :cols], ACTF.Exp,
                            bias=negB[m][:], scale=1.0,
                            accum_out=Sh[m][:, s + sub:s + sub + 1],
                        )
                    if s == ST - 1 and m == 0:
                        # m0 complete: its sum/recip/norms/out-DMA all
                        # hide under m1's final chunk + tail. All norms
                        # on DVE: an ACT COPY here would block m1's
                        # final exp in the ACT queue.
                        rescale(0)
                        for ss in range(ST):
                            norm_chunk(0, ss, SB, nc.vector)
                        out_pair(0, 0, nc.gpsimd)
                        out_pair(0, 2, nc.sync)
                        out_pair(0, 4, nc.scalar)
                        out_pair(0, 6, nc.gpsimd)

            rescale(1)
            for ss in range(6):
                norm_chunk(1, ss, SB, nc.vector)
            norm_chunk(1, 6, SB, nc.scalar)
            norm_chunk(1, 7, SB, nc.vector)
            out_pair(1, 0, nc.sync)
            out_pair(1, 2, nc.scalar)
            out_pair(1, 4, nc.gpsimd)
            # final pieces as singles on separate rings: shorter last-DMA
            # latency after the last norm
            nc.scalar.dma_start(
                out[P:2 * P, SB * 6:SB * 7], out_sb[:, 1, SB * 6:SB * 7])
            nc.sync.dma_start(
                out[P:2 * P, SB * 7:SB * 8], out_sb[:, 1, SB * 7:SB * 8])

    nc.compile()
    return nc


def _prepare(user_emb, id_emb, socail_uid_emb, attn_W, w1, w2):
    """Host-side sharding + packing. Returns (ratio, swap, in_maps).

    Packed layouts (per-partition contiguous runs -> few big DMA
    descriptors):
      usT: [128, KT, 2, ROWS]  [p,k,0,m] = in0[rows0+m, k*128+p]
                               [p,k,1,m] = in1[rows0+m, k*128+p]  (fp16)
      Wp:  [128, KT, HID]      [p,h,c] = wbig*W[h*128+p, c]       (fp16)
      idT: [ST, 128, KT, SB]   [s,p,k,c] = id[s*512+c, k*128+p]   (fp16)
    where in0 is the smaller-|w| side (scaled by ratio on device) and
    in1 the larger side.
    """
    w1 = float(np.asarray(w1))
    w2 = float(np.asarray(w2))
    swap = abs(w2) > abs(w1)
    wbig = w2 if swap else w1
    wsmall = w1 if swap else w2
    ratio = (wsmall / wbig) if wbig != 0.0 else 0.0

    Wp = (np.float32(wbig) * np.asarray(attn_W, np.float32)).astype(np.float16)
    Wp_pack = np.ascontiguousarray(Wp.reshape(KT, P, HID).transpose(1, 0, 2))

    idh = np.asarray(id_emb, np.float32).astype(np.float16)      # [SEQ, HID]
    idT_pack = np.ascontiguousarray(
        idh.reshape(ST, SB, KT, P).transpose(0, 3, 2, 1)         # [s,p,k,c]
    )

    u = np.asarray(user_emb, np.float32).astype(np.float16)
    s_ = np.asarray(socail_uid_emb, np.float32).astype(np.float16)
    in0_full = s_ if not swap else u       # scaled by ratio on device
    in1_full = u if not swap else s_

    in_maps = []
    for i in range(NCORES):
        rows = slice(ROWS * i, ROWS * (i + 1))
        us = np.empty((P, KT, 2, ROWS), np.float16)
        us[:, :, 0, :] = in0_full[rows].reshape(ROWS, KT, P).transpose(2, 1, 0)
        us[:, :, 1, :] = in1_full[rows].reshape(ROWS, KT, P).transpose(2, 1, 0)
        in_maps.append({
            "usT": np.ascontiguousarray(us),
            "Wp": Wp_pack,
            "idT": idT_pack,
        })
    return ratio, swap, in_maps


def kernel(user_emb, id_emb, socail_uid_emb, attn_W, attn_b, w1, w2):
    from concourse.bass_utils import run_bass_kernel_spmd

    ratio, swap, in_maps = _prepare(user_emb, id_emb, socail_uid_emb, attn_W, w1, w2)

    key = (round(ratio, 9), swap)
    nc = _graph_cache.get(key)
    if nc is None:
        nc = _build(ratio, swap)
        _graph_cache[key] = nc

    res = run_bass_kernel_spmd(nc, in_maps, core_ids=list(range(NCORES)))
    return np.concatenate(
        [res.results[i]["out"].astype(np.float32) for i in range(NCORES)], axis=0)
